# revision 1
# baseline (speedup 1.0000x reference)
"""TRN2 Bass kernel for nn_AttnPlainNet (gnn_message_passing).

Math (C=1 collapses everything):
  l2norm over C=1  -> u = sign(x), sgn_nb = sign(neighbor)
  att weights      -> watt[b,n] = softmax_n(s_x[b]*s_y[b,n])
  v[b,f] = sum_n watt*sgn_nb ; w = u*v
  fadj[a,e] = u_a u_e S(w_a+w_e) / (d_e + eps),  S(t)=sign(t)sqrt|t|,
  d_e = sum_a sqrt|w_a+w_e|   (A = S-matrix is symmetric)
  layer1: z1[k] = u_k t_k/(d_k+eps), t_k = sum_f S(w_f+w_k)
  BN1 is affine in z1 (stats from global z1 mean/var -> 2-float all-reduce)
  p~ = softsign(alpha*z1+beta)*u ; layer2: z2[k,c] = u_k/(d_k+eps) *
        sum_f As[f,k] p~[f,c]  (PE matmul over cached As)
  BN2 stats from z2 first/second moments (16x17 all-reduce)
  q = softsign(W2' z2 + delta) ; out = q @ WcT + bc
Sharding: pure data-parallel, 32 batches per core, 8 cores.
"""
from contextlib import ExitStack

import numpy as np

import concourse.bass as bass
import concourse.mybir as mybir
import concourse.tile as tile
from concourse import bacc
from concourse.bass_utils import run_bass_kernel_spmd
from concourse.masks import make_identity

# Steer the act-table-set chooser away from the partial ln-only / exp-only
# sets so Ln+Exp sequences stay resident in natural_log_exp_and_others
# (positional set ids must be preserved, so entries are emptied, not removed).
_orig_get_tables = bacc.get_activation_tables


def _patched_get_tables(arch):
    tabs = dict(_orig_get_tables(arch))
    for name in ("natural_log", "exp_and_others", "exp_and_friends"):
        if name in tabs:
            tabs[name] = set()
    return tabs


bacc.get_activation_tables = _patched_get_tables

AF = mybir.ActivationFunctionType
ALU = mybir.AluOpType
F32 = mybir.dt.float32
F16 = mybir.dt.float16
U16 = mybir.dt.uint16

B, N, F, H, NCLS = 256, 32, 512, 16, 64
NCORES = 8
BL = B // NCORES          # 32 local batches
FC = 4                    # f/k chunks of 128
P = 128
EPS_ROW = 1e-7
EPS_BN = 1e-5
NK = float(B * F)         # BN normalizer (global)

_CACHE = {}


def _bc_ap(handle_ap, ap):
    """AP with explicit [stride, count] dims over a tensor handle's AP."""
    return bass.AP(tensor=handle_ap.tensor, offset=handle_ap.offset, ap=ap)


def build_program(no_cc=False):
    nc = bacc.Bacc("TRN2", num_devices=NCORES)

    # ---- I/O -------------------------------------------------------------
    x_l = nc.dram_tensor("x_l", [BL, F], F32, kind="ExternalInput")
    nb_l = nc.dram_tensor("nb_l", [BL * N, F], F32, kind="ExternalInput")
    att1 = nc.dram_tensor("att1", [1, F], F32, kind="ExternalInput")
    att2 = nc.dram_tensor("att2", [1, F], F32, kind="ExternalInput")
    w1c = nc.dram_tensor("w1c", [H, 1], F32, kind="ExternalInput")
    b1 = nc.dram_tensor("b1", [H, 1], F32, kind="ExternalInput")
    g1 = nc.dram_tensor("g1", [H, 1], F32, kind="ExternalInput")
    be1 = nc.dram_tensor("be1", [H, 1], F32, kind="ExternalInput")
    w2 = nc.dram_tensor("w2", [H, H], F32, kind="ExternalInput")
    w2t = nc.dram_tensor("w2t", [H, H], F32, kind="ExternalInput")
    b2 = nc.dram_tensor("b2", [H, 1], F32, kind="ExternalInput")
    g2 = nc.dram_tensor("g2", [H, 1], F32, kind="ExternalInput")
    be2 = nc.dram_tensor("be2", [H, 1], F32, kind="ExternalInput")
    wct = nc.dram_tensor("wct", [H * F, NCLS], F16, kind="ExternalInput")
    bc = nc.dram_tensor("bc", [1, NCLS], F32, kind="ExternalInput")
    out_l = nc.dram_tensor("out_l", [BL, NCLS], F32, kind="ExternalOutput")

    with tile.TileContext(nc) as tc, ExitStack() as ctx:
        sg = ctx.enter_context(tc.tile_pool(name="singles", bufs=1))
        wk = ctx.enter_context(tc.tile_pool(name="work", bufs=2))
        t2 = ctx.enter_context(tc.tile_pool(name="t2", bufs=2))
        bigp = ctx.enter_context(tc.tile_pool(name="big2", bufs=1))
        wbp = ctx.enter_context(tc.tile_pool(name="wbp", bufs=3))
        st1ctx = ExitStack()
        s1 = st1ctx.enter_context(tc.tile_pool(name="stage1", bufs=1))
        dr = ctx.enter_context(tc.tile_pool(name="dram", bufs=1, space="DRAM"))
        ps = ctx.enter_context(tc.tile_pool(name="psmall", bufs=2, space="PSUM"))
        pgt = ctx.enter_context(tc.tile_pool(name="pgt", bufs=2, space="PSUM"))
        pm2 = ctx.enter_context(tc.tile_pool(name="pm2", bufs=1, space="PSUM"))
        pq = ctx.enter_context(tc.tile_pool(name="pq", bufs=1, space="PSUM"))

        V, S, G = nc.vector, nc.scalar, nc.gpsimd
        TE = nc.tensor

        # ---- constants ---------------------------------------------------
        i32 = sg.tile([32, 32], F32)
        make_identity(nc, i32[:])
        i32h = sg.tile([32, 32], F16)
        make_identity(nc, i32h[:])
        i16 = sg.tile([16, 16], F32)
        make_identity(nc, i16[:])
        i16h = sg.tile([16, 16], F16)
        make_identity(nc, i16h[:])
        i128h = sg.tile([P, P], F16)
        make_identity(nc, i128h[:])
        epsb = sg.tile([H, 1], F32)
        V.memset(epsb[:], EPS_BN)
        ones128 = sg.tile([P, 1], F32)
        V.memset(ones128[:], 1.0)
        ones128h = sg.tile([P, 1], F16)
        V.memset(ones128h[:], 1.0)
        onesrow = sg.tile([1, P], F32)
        V.memset(onesrow[:], 1.0)
        blkones = sg.tile([P, 4], F32)
        V.memset(blkones[:], 0.0)
        for a in range(4):
            V.memset(blkones[32 * a:32 * a + 32, a:a + 1], 1.0)

        # broadcast att vectors
        att1_b = s1.tile([32, F], F32)
        nc.sync.dma_start(att1_b[:], _bc_ap(att1[:], [[0, 32], [1, F]]))
        att2_b = s1.tile([P, F], F32)
        nc.sync.dma_start(att2_b[:], _bc_ap(att2[:], [[0, P], [1, F]]))

        # WcT tiles [128, 64jc, 64n] fp16
        wct_sb = sg.tile([P, 64, NCLS], F16)
        nc.sync.dma_start(wct_sb[:], wct[:].rearrange("(jc p) n -> p jc n", p=P))
        bc_rep = sg.tile([8, NCLS], F32)
        nc.sync.dma_start(bc_rep[:], _bc_ap(bc[:], [[0, 8], [1, NCLS]]))

        # per-channel weights [16,1]
        w1s = sg.tile([H, 1], F32)
        nc.sync.dma_start(w1s[:], w1c[:])
        b1s = sg.tile([H, 1], F32)
        nc.sync.dma_start(b1s[:], b1[:])
        g1s = sg.tile([H, 1], F32)
        nc.sync.dma_start(g1s[:], g1[:])
        be1s = sg.tile([H, 1], F32)
        nc.sync.dma_start(be1s[:], be1[:])
        b2s = sg.tile([H, 1], F32)
        nc.sync.dma_start(b2s[:], b2[:])
        g2s = sg.tile([H, 1], F32)
        nc.sync.dma_start(g2s[:], g2[:])
        be2s = sg.tile([H, 1], F32)
        nc.sync.dma_start(be2s[:], be2[:])
        w2s = sg.tile([H, H], F32)
        nc.sync.dma_start(w2s[:], w2[:])
        w2ts = sg.tile([H, H], F32)
        nc.sync.dma_start(w2ts[:], w2t[:])

        # ---- stage 0: x -> u, s_x ---------------------------------------
        xsb = wk.tile([P, F], F32, tag="nbt")
        nc.sync.dma_start(xsb[0:BL, :], x_l[:])
        u32 = sg.tile([BL, F], F32)
        S.activation(u32[:], xsb[0:BL, :], AF.Sign)
        sx_col = sg.tile([BL, 1], F32)
        V.scalar_tensor_tensor(xsb[0:BL, :], u32[:], 0.0, att1_b[:],
                               ALU.bypass, ALU.mult, accum_out=sx_col[:])

        # ---- stage 1 (pipelined per tile): sgn, s_y, softmax, v, w ----
        i4 = sg.tile([4, 4], F32)
        make_identity(nc, i4[:])
        sx_d = dr.tile([BL], F32)
        nc.sync.dma_start(sx_d[:], sx_col[:].rearrange("b one -> (b one)"))
        sx_rep = sg.tile([P, 8], F32)
        for a in range(4):
            nc.sync.dma_start(sx_rep[32 * a:32 * a + 32, :],
                        bass.AP(tensor=sx_d[:].tensor,
                                offset=sx_d[:].offset + a,
                                ap=[[0, 32], [4, 8]]))
        w16_ds = [dr.tile([4, F], F16, tag=f"w16d{j}", name=f"w16d{j}") for j in range(8)]
        wT_js = [sg.tile([P, 16], F32, tag=f"wtj{j}", name=f"wtj{j}") for j in range(8)]
        for j in range(8):
            nbt = wk.tile([P, F], F32, tag="nbt")
            nc.sync.dma_start(nbt[:], nb_l[:].rearrange("(j p) f -> j p f", p=P)[j])
            sgn = wk.tile([P, F], F32, tag="sgn")
            S.activation(sgn[:], nbt[:], AF.Sign)
            sy = wk.tile([P, 1], F32, tag="sy")
            V.scalar_tensor_tensor(nbt[:], sgn[:], 0.0, att2_b[:],
                                   ALU.bypass, ALU.mult, accum_out=sy[:])
            lcol = wk.tile([P, 1], F32, tag="lcol")
            V.tensor_tensor(lcol[:], sy[:], sx_rep[:, j:j + 1], ALU.mult)
            ecol = wk.tile([P, 1], F32, tag="ecol")
            S.activation(ecol[:], lcol[:], AF.Exp)
            p_dn = ps.tile([4, 1], F32, tag="sm")
            TE.matmul(p_dn[:], blkones[:], ecol[:], start=True, stop=True)
            rdn = wk.tile([4, 1], F32, tag="rdn")
            V.reciprocal(rdn[:], p_dn[:])
            wd4 = wk.tile([P, 4], F32, tag="wd")
            V.tensor_tensor(wd4[:], ecol[:].to_broadcast([P, 4]),
                            blkones[:], ALU.mult)
            p_vj = ps.tile([4, F], F32, tag="sm")
            TE.matmul(p_vj[:], wd4[:], sgn[:], start=True, stop=True)
            u_j = wk.tile([4, F], F32, tag="uj")
            nc.sync.dma_start(u_j[:], u32[4 * j:4 * j + 4, :])
            w_j = wk.tile([4, F], F32, tag="wj")
            V.tensor_scalar(w_j[:], p_vj[:], rdn[:], None, ALU.mult)
            V.tensor_tensor(w_j[:], w_j[:], u_j[:], ALU.mult)
            w16_j = wk.tile([4, F], F16, tag="w16j")
            V.tensor_copy(w16_j[:], w_j[:])
            nc.sync.dma_start(w16_ds[j][:], w16_j[:])
            p_wt = ps.tile([P, 4, 4], F32, tag="sm")
            for c in range(FC):
                TE.transpose(p_wt[:, c, :], w_j[:, P * c:P * c + P], i4[:])
            V.tensor_copy(wT_js[j][:], p_wt[:])

        # u transpose (for BN1/ptil later)
        p_tu = ps.tile([P, P], F32, tag="sm")
        for c in range(FC):
            TE.transpose(p_tu[:, 32 * c:32 * c + 32],
                         u32[:, P * c:P * c + P], i32[:])
        uT = sg.tile([P, P], F32)
        V.tensor_copy(uT[:], p_tu[:])
        st1ctx.close()

        # ---- stage 2: main pass-1 loop (A matrix, d, t, As cache) --------
        as_cache = sg.tile([P, FC, BL, F], F16)
        onehot = sg.tile([P, 63], F16)
        V.memset(onehot[:], 0.0)
        V.memset(onehot[:, 31:32], 1.0)
        p_t32 = pm2.tile([BL, F], F32, tag="pm2")
        p_d32 = pm2.tile([BL, F], F32, tag="pm1")
        for b in range(BL):
            w_bc = wbp.tile([P, F], F16, tag="wbc")
            wd_ap = w16_ds[b // 4][:]
            nc.sync.dma_start(w_bc[:], bass.AP(tensor=wd_ap.tensor,
                                         offset=wd_ap.offset + (b % 4) * F,
                                         ap=[[0, P], [1, F]]))
            t4 = t2.tile([P, FC, F], F16, tag="T")
            for c in range(FC):
                wtj = wT_js[b // 4]
                V.tensor_scalar(t4[:, c, :], w_bc[:],
                                wtj[:, 4 * c + b % 4:4 * c + b % 4 + 1],
                                None, ALU.add)
            sig4 = t2.tile([P, FC, F], F16, tag="sig")
            V.tensor_scalar(sig4[:].bitcast(U16), t4[:].bitcast(U16),
                            0x8000, 0x3C00, ALU.bitwise_and, ALU.bitwise_or)
            V.tensor_scalar(t4[:].bitcast(U16), t4[:].bitcast(U16),
                            0x7FFF, None, ALU.bitwise_and)
            r4 = t2.tile([P, FC, F], F16, tag="r")
            S.activation(r4[:], t4[:], AF.Sqrt)
            V.tensor_tensor(as_cache[:, :, b, :], sig4[:], r4[:], ALU.mult)
            oh = onehot[:, 31 - b:63 - b]
            for c in range(FC):
                TE.matmul(p_t32[:], oh, as_cache[:, c, b, :],
                          start=(b == 0 and c == 0),
                          stop=(b == BL - 1 and c == FC - 1))
                TE.matmul(p_d32[:], oh, r4[:, c, :],
                          start=(b == 0 and c == 0),
                          stop=(b == BL - 1 and c == FC - 1))
        t_rows = sg.tile([BL, F], F16)
        V.tensor_copy(t_rows[:], p_t32[:])
        d_rows = sg.tile([BL, F], F16)
        V.tensor_copy(d_rows[:], p_d32[:])
        p_tt = ps.tile([P, P], F16, tag="sm")
        for c in range(FC):
            TE.transpose(p_tt[:, 32 * c:32 * c + 32],
                         t_rows[:, P * c:P * c + P], i32h[:])
        tT = sg.tile([P, P], F32)
        V.tensor_copy(tT[:], p_tt[:])
        p_dd = ps.tile([P, P], F16, tag="sm")
        for c in range(FC):
            TE.transpose(p_dd[:, 32 * c:32 * c + 32],
                         d_rows[:, P * c:P * c + P], i32h[:])
        dT = sg.tile([P, P], F32)
        V.tensor_copy(dT[:], p_dd[:])

        # ---- BN1 stats + all-reduce --------------------------------------
        V.tensor_scalar(dT[:], dT[:], EPS_ROW, None, ALU.add)
        recdT = sg.tile([P, P], F32)
        V.reciprocal(recdT[:], dT[:])
        urdT = sg.tile([P, P], F32)
        V.tensor_tensor(urdT[:], uT[:], recdT[:], ALU.mult)
        z1T = sg.tile([P, P], F32)
        V.tensor_tensor(z1T[:], tT[:], urdT[:], ALU.mult)
        z1sq = t2.tile([P, P], F32, tag="r")
        V.tensor_tensor(z1sq[:], z1T[:], z1T[:], ALU.mult)
        rs = sg.tile([P, 2], F32)
        V.reduce_sum(rs[:, 0:1], z1T[:], axis=mybir.AxisListType.X)
        V.reduce_sum(rs[:, 1:2], z1sq[:], axis=mybir.AxisListType.X)
        p_s = ps.tile([1, 2], F32, tag="sm")
        TE.matmul(p_s[:], ones128[:], rs[:], start=True, stop=True)
        s_loc = sg.tile([1, 2], F32)
        V.tensor_copy(s_loc[:], p_s[:])
        cc1_in = dr.tile([1, 2], F32)
        cc1_out = dr.tile([1, 2], F32)
        nc.sync.dma_start(cc1_in[:], s_loc[:])
        if no_cc:
            nc.sync.dma_start(cc1_out[:], cc1_in[:])
        else:
            G.collective_compute("AllReduce", ALU.add,
                                 replica_groups=[list(range(NCORES))],
                                 ins=[cc1_in[:].opt()],
                                 outs=[cc1_out[:].opt()])
        sg_b = sg.tile([H, 2], F32)
        nc.sync.dma_start(sg_b[:], _bc_ap(cc1_out[:], [[0, H], [1, 2]]))

        # per-channel BN1 affine params
        mz = sg.tile([H, 1], F32)
        V.tensor_scalar(mz[:], sg_b[:, 0:1], 1.0 / NK, None, ALU.mult)
        e2m = sg.tile([H, 1], F32)
        V.tensor_scalar(e2m[:], sg_b[:, 1:2], 1.0 / NK, None, ALU.mult)
        tmp = sg.tile([H, 1], F32)
        V.tensor_tensor(tmp[:], mz[:], mz[:], ALU.mult)
        varz = sg.tile([H, 1], F32)
        V.tensor_tensor(varz[:], e2m[:], tmp[:], ALU.subtract)
        w1sq = sg.tile([H, 1], F32)
        V.tensor_tensor(w1sq[:], w1s[:], w1s[:], ALU.mult)
        var1 = sg.tile([H, 1], F32)
        V.tensor_tensor(var1[:], w1sq[:], varz[:], ALU.mult)
        invsd = sg.tile([H, 1], F32)
        S.activation(invsd[:], var1[:], AF.Ln, bias=epsb[:])
        S.activation(invsd[:], invsd[:], AF.Exp, scale=-0.5)
        alpha = sg.tile([H, 1], F32)
        V.tensor_tensor(alpha[:], w1s[:], g1s[:], ALU.mult)
        V.tensor_tensor(alpha[:], alpha[:], invsd[:], ALU.mult)
        m1 = sg.tile([H, 1], F32)
        V.tensor_tensor(m1[:], w1s[:], mz[:], ALU.mult)
        V.tensor_tensor(m1[:], m1[:], b1s[:], ALU.add)
        beta = sg.tile([H, 1], F32)
        V.tensor_tensor(beta[:], b1s[:], m1[:], ALU.subtract)
        V.tensor_tensor(beta[:], beta[:], g1s[:], ALU.mult)
        V.tensor_tensor(beta[:], beta[:], invsd[:], ALU.mult)
        V.tensor_tensor(beta[:], beta[:], be1s[:], ALU.add)

        p_ab = ps.tile([1, 2 * H], F32, tag="sm")
        TE.transpose(p_ab[:, 0:H], alpha[:], i16[:])
        TE.transpose(p_ab[:, H:2 * H], beta[:], i16[:])
        ab_row = sg.tile([1, 2 * H], F32)
        V.tensor_copy(ab_row[:], p_ab[:])
        p_abb = ps.tile([P, 2 * H], F32, tag="sm")
        TE.matmul(p_abb[:, 0:H], onesrow[:], ab_row[0:1, 0:H],
                  start=True, stop=True)
        TE.matmul(p_abb[:, H:2 * H], onesrow[:], ab_row[0:1, H:2 * H],
                  start=True, stop=True)
        abb = sg.tile([P, 2 * H], F32)
        V.tensor_copy(abb[:], p_abb[:])
        alpha_b = abb[:, 0:H]
        beta_b = abb[:, H:2 * H]

        # ---- p~ = softsign(alpha*z1+beta)*u  (fp16, [128, 128cb*16]) -----
        sfull = t2.tile([P, P, H], F16, tag="T")
        absS = t2.tile([P, P, H], F16, tag="sig")
        ptil = bigp.tile([P, P, H], F16, tag="big")
        HH = P // 2
        for h in range(2):
            sl = slice(h * HH, (h + 1) * HH)
            V.tensor_tensor(sfull[:, sl, :],
                            z1T[:, sl, None].to_broadcast([P, HH, H]),
                            alpha_b[:, None, :].to_broadcast([P, HH, H]),
                            ALU.mult)
            V.tensor_tensor(sfull[:, sl, :], sfull[:, sl, :],
                            beta_b[:, None, :].to_broadcast([P, HH, H]),
                            ALU.add)
            S.activation(absS[:, sl, :], sfull[:, sl, :], AF.Abs)
            S.activation(absS[:, sl, :], absS[:, sl, :], AF.Ln, bias=1.0)
            S.activation(absS[:, sl, :], absS[:, sl, :], AF.Exp, scale=-1.0)
            V.tensor_tensor(ptil[:, sl, :], sfull[:, sl, :], absS[:, sl, :],
                            ALU.mult)
            V.tensor_tensor(ptil[:, sl, :], ptil[:, sl, :],
                            uT[:, sl, None].to_broadcast([P, HH, H]),
                            ALU.mult)

        # ---- pass 2: GT matmuls, z2, M1/M2 -------------------------------
        z2T = sg.tile([P, FC, BL, H], F16)
        for g in range(4):
            p_gt = pgt.tile([P, FC, 8, H], F32, tag="pgt")
            for bb in range(8):
                b = 8 * g + bb
                for kc in range(FC):
                    for fc in range(FC):
                        TE.matmul(p_gt[:, kc, bb, :],
                                  as_cache[:, fc, b, P * kc:P * kc + P],
                                  ptil[:, fc * 32 + b, :],
                                  start=(fc == 0), stop=(fc == FC - 1))
            u4 = urdT[:].rearrange("p (c b) -> p c b", c=FC)
            V.tensor_tensor(
                z2T[:, :, 8 * g:8 * g + 8, :], p_gt[:],
                u4[:, :, 8 * g:8 * g + 8, None].to_broadcast([P, FC, 8, H]),
                ALU.mult)

        p_m2 = pm2.tile([H, H], F32, tag="pm2")
        p_m1 = pm2.tile([1, H], F32, tag="pm1")
        for cb in range(FC * BL):
            kc, b = divmod(cb, BL)
            TE.matmul(p_m2[:], z2T[:, kc, b, :], z2T[:, kc, b, :],
                      start=(cb == 0), stop=(cb == FC * BL - 1))
        for cb in range(FC * BL):
            kc, b = divmod(cb, BL)
            TE.matmul(p_m1[:], ones128h[:], z2T[:, kc, b, :],
                      start=(cb == 0), stop=(cb == FC * BL - 1))
        m2_sb = sg.tile([H, H], F32)
        V.tensor_copy(m2_sb[:], p_m2[:])
        m1_sb = sg.tile([1, H], F32)
        V.tensor_copy(m1_sb[:], p_m1[:])
        cc2_in = dr.tile([H + 1, H], F32)
        cc2_out = dr.tile([H + 1, H], F32)
        nc.sync.dma_start(cc2_in[0:H, :], m2_sb[:])
        nc.sync.dma_start(cc2_in[H:H + 1, :], m1_sb[:])
        if no_cc:
            nc.sync.dma_start(cc2_out[:], cc2_in[:])
        else:
            G.collective_compute("AllReduce", ALU.add,
                                 replica_groups=[list(range(NCORES))],
                                 ins=[cc2_in[:].opt()],
                                 outs=[cc2_out[:].opt()])
        m2g = sg.tile([H, H], F32)
        nc.sync.dma_start(m2g[:], cc2_out[0:H, :])
        m1_b = sg.tile([H, H], F32)
        c2ap = cc2_out[:]
        nc.sync.dma_start(m1_b[:], bass.AP(tensor=c2ap.tensor,
                                     offset=c2ap.offset + H * H,
                                     ap=[[0, H], [1, H]]))

        # ---- BN2 affine params -------------------------------------------
        p_a1 = ps.tile([H, H], F32, tag="sm")
        TE.matmul(p_a1[:], w2ts[:], m2g[:], start=True, stop=True)
        a1 = sg.tile([H, H], F32)
        V.tensor_copy(a1[:], p_a1[:])
        t16 = sg.tile([H, H], F32)
        V.tensor_tensor(t16[:], a1[:, 0:H], w2s[:], ALU.mult)
        diagq = sg.tile([H, 1], F32)
        V.reduce_sum(diagq[:], t16[:], axis=mybir.AxisListType.X)
        wm1t = sg.tile([H, H], F32)
        V.tensor_tensor(wm1t[:], w2s[:], m1_b[:], ALU.mult)
        wm1 = sg.tile([H, 1], F32)
        V.reduce_sum(wm1[:], wm1t[:], axis=mybir.AxisListType.X)
        m2o = sg.tile([H, 1], F32)
        V.tensor_scalar(m2o[:], wm1[:], 1.0 / NK, None, ALU.mult)
        V.tensor_tensor(m2o[:], m2o[:], b2s[:], ALU.add)
        eh2 = sg.tile([H, 1], F32)
        V.tensor_scalar(eh2[:], diagq[:], 1.0 / NK, None, ALU.mult)
        tb2 = sg.tile([H, 1], F32)
        V.tensor_tensor(tb2[:], b2s[:], wm1[:], ALU.mult)
        V.tensor_scalar(tb2[:], tb2[:], 2.0 / NK, None, ALU.mult)
        V.tensor_tensor(eh2[:], eh2[:], tb2[:], ALU.add)
        b2sq = sg.tile([H, 1], F32)
        V.tensor_tensor(b2sq[:], b2s[:], b2s[:], ALU.mult)
        V.tensor_tensor(eh2[:], eh2[:], b2sq[:], ALU.add)
        m2sq = sg.tile([H, 1], F32)
        V.tensor_tensor(m2sq[:], m2o[:], m2o[:], ALU.mult)
        var2 = sg.tile([H, 1], F32)
        V.tensor_tensor(var2[:], eh2[:], m2sq[:], ALU.subtract)
        invsd2 = sg.tile([H, 1], F32)
        S.activation(invsd2[:], var2[:], AF.Ln, bias=epsb[:])
        S.activation(invsd2[:], invsd2[:], AF.Exp, scale=-0.5)
        gam = sg.tile([H, 1], F32)
        V.tensor_tensor(gam[:], g2s[:], invsd2[:], ALU.mult)
        w2p = sg.tile([H, H], F16)
        V.tensor_scalar(w2p[:], w2s[:], gam[:], None, ALU.mult)
        delta = sg.tile([H, 1], F32)
        V.tensor_tensor(delta[:], b2s[:], m2o[:], ALU.subtract)
        V.tensor_tensor(delta[:], delta[:], gam[:], ALU.mult)
        V.tensor_tensor(delta[:], delta[:], be2s[:], ALU.add)

        p_w2p = ps.tile([H, H], F16, tag="sm")
        TE.transpose(p_w2p[:], w2p[:], i16h[:])
        w2pt = sg.tile([H, H], F16)
        V.tensor_copy(w2pt[:], p_w2p[:])
        bd = sg.tile([P, P], F16)
        V.memset(bd[:], 0.0)
        w2pt_d = dr.tile([H, H], F16)
        nc.sync.dma_start(w2pt_d[:], w2pt[:])
        for i in range(8):
            nc.sync.dma_start(bd[16 * i:16 * i + 16, 16 * i:16 * i + 16],
                        w2pt_d[:])
        i16big = sg.tile([H, P], F32)
        for i in range(8):
            V.tensor_copy(i16big[:, H * i:H * i + H], i16[:])
        p_dl = ps.tile([P, 1], F32, tag="sm")
        TE.matmul(p_dl[:], i16big[:], delta[:], start=True, stop=True)
        dl_rep = sg.tile([P, 1], F32)
        V.tensor_copy(dl_rep[:], p_dl[:])

        # ---- q phase + classifier ----
        qt_all = bigp.tile([P, 4, FC, P], F16, tag="big")
        qs_all = t2.tile([P, 4, F], F16, tag="T")
        for g in range(4):
            pp = pq if g % 2 == 0 else pm2
            p_z2c = pp.tile([P, F], F16, tag="pm2" if g % 2 else "pz2c",
                            name=f"pz2c{g}")
            for kc in range(FC):
                TE.transpose(p_z2c[:, P * kc:P * kc + P],
                             z2T[:, kc, 8 * g:8 * g + 8, :], i128h[:])
            z2c = wk.tile([P, F], F16, tag="z2c")
            V.tensor_copy(z2c[:], p_z2c[:])
            p_q = pp.tile([P, F], F32, tag="pm1" if g % 2 else "pqm",
                          name=f"pqm{g}")
            TE.matmul(p_q[:], bd[:], z2c[:], start=True, stop=True)
            V.tensor_scalar(qs_all[:, g, :], p_q[:], dl_rep[:], None, ALU.add)
        rq_all = t2.tile([P, 4, F], F16, tag="sig")
        q8_all = t2.tile([P, 4, F], F16, tag="r")
        for h in range(2):
            sl = slice(h * 2, (h + 1) * 2)
            S.activation(rq_all[:, sl, :], qs_all[:, sl, :], AF.Abs)
            S.activation(rq_all[:, sl, :], rq_all[:, sl, :], AF.Ln, bias=1.0)
            S.activation(rq_all[:, sl, :], rq_all[:, sl, :], AF.Exp,
                         scale=-1.0)
            V.tensor_tensor(q8_all[:, sl, :], qs_all[:, sl, :],
                            rq_all[:, sl, :], ALU.mult)
        for g in range(4):
            for kc in range(FC):
                nc.sync.dma_start_transpose(qt_all[:, g, kc, :],
                                            q8_all[:, g, P * kc:P * kc + P])
        for g in range(4):
            p_o = ps.tile([8, NCLS], F32, tag="sm")
            for o in range(H):
                for kc in range(FC):
                    jc = o * FC + kc
                    TE.matmul(p_o[:],
                              qt_all[:, g, kc, o:P:H],
                              wct_sb[:, jc, :],
                              start=(jc == 0), stop=(jc == H * FC - 1))
            out_f = wk.tile([8, NCLS], F32, tag="outf")
            V.tensor_tensor(out_f[:], p_o[:], bc_rep[:], ALU.add)
            nc.sync.dma_start(out_l[:].rearrange("(g e) n -> g e n", g=4)[g],
                        out_f[:])

    nc.finalize()
    return nc


def kernel(**inputs):
    x = np.asarray(inputs["x"], np.float32)            # [256,1,512]
    nb = np.asarray(inputs["neighbor"], np.float32)    # [256,32,1,512]
    if "prog" not in _CACHE:
        _CACHE["prog"] = build_program()
    nc = _CACHE["prog"]

    shared = {
        "att1": np.ascontiguousarray(
            np.asarray(inputs["att1_w"], np.float32)[None, :]),
        "att2": np.ascontiguousarray(
            np.asarray(inputs["att2_w"], np.float32)[None, :]),
        "w1c": np.ascontiguousarray(np.asarray(inputs["W1"], np.float32)),
        "b1": np.asarray(inputs["b1"], np.float32)[:, None].copy(),
        "g1": np.asarray(inputs["g1"], np.float32)[:, None].copy(),
        "be1": np.asarray(inputs["be1"], np.float32)[:, None].copy(),
        "w2": np.ascontiguousarray(np.asarray(inputs["W2"], np.float32)),
        "w2t": np.ascontiguousarray(np.asarray(inputs["W2"],
                                               np.float32).T),
        "b2": np.asarray(inputs["b2"], np.float32)[:, None].copy(),
        "g2": np.asarray(inputs["g2"], np.float32)[:, None].copy(),
        "be2": np.asarray(inputs["be2"], np.float32)[:, None].copy(),
        "wct": np.ascontiguousarray(
            np.asarray(inputs["Wc"], np.float32).T.astype(np.float16)),
        "bc": np.ascontiguousarray(
            np.asarray(inputs["bc"], np.float32)[None, :]),
    }
    in_maps = []
    for c in range(NCORES):
        sl = slice(c * BL, (c + 1) * BL)
        m = dict(shared)
        m["x_l"] = np.ascontiguousarray(x[sl, 0, :])
        m["nb_l"] = np.ascontiguousarray(
            nb[sl, :, 0, :].reshape(BL * N, F))
        in_maps.append(m)

    res = run_bass_kernel_spmd(nc, in_maps, core_ids=list(range(NCORES)))
    return np.concatenate([r["out_l"] for r in res.results], axis=0)

